# revision 37
# baseline (speedup 1.0000x reference)
"""Trainium2 Bass kernel for nn_ConvLayer_13967233646751 (gnn_message_passing).

Design (~146-148us on 8 cores, rel err 3.0e-3; baseline was 161us):
  - data-parallel over batch B=16 across 8 cores (2 batches/core)
  - host packs fp16: feats (64ch), geo (9ch, [p,q,j,t,s]), w1 column-
    permuted so GEMM1 is a single 73-row matmul per 512 columns.
  - R phase: rotation matrices from normals + azimuth means; xyz sums as
    elementwise adds (no mid-chain DVE reductions), sqrt on ACT (shares a
    table set with relu -> exactly one ACT_TABLE_LOAD), reciprocal on DVE.
    b0's R phase + first two rotation groups on DVE (idle at startup);
    b1's R phase elementwise + remaining rotations on gpsimd, all
    front-loaded before the chunk loop.
  - aligned geo goes p-major -> c-major via a DRAM round trip: scatter
    write (64B descriptors) + contiguous read, both on the sync HWDGE
    ring (bulk feats/geo loads live on the scalar HWDGE ring; the gpsimd
    SWDGE queue does no DMA at all, avoiding its in-flight slot limit).
  - chunk loop (64 x 1024 cols): PE mm1 (73x128) software-pipelined one
    chunk ahead of mm2 (128x128), ACT relu+bias evacuation, DVE max-pool
    from PSUM, pooled relu + outp DMA per 256 columns.
  - 8 warmup matmuls gated on the first rotation output keep the HAM
    clock gate warm into chunk 0; a dummy relu at t~2 hoists the one-time
    ACT_TABLE_LOAD off the R-phase critical path.
  - ring discipline (the big win, -14us): all bulk loads emitted first on
    the scalar ring; g0's rt round trip split q0-on-sync / q1-on-scalar
    (parallel rings); b1 rotation in nq=2 groups; outa on scalar at the
    end.  Rule: a DMA that will WAIT on a semaphore must never sit on the
    scalar/ACT ring while relu evacs run — it blocks the ACT queue head.

Hard-won notes (measured this session):
  - tile_position / small-K split matmuls keep the PE at K=4/8 half
    clock for the whole run (HAM never warms) -> split-GEMM1 designs and
    the xbar-transpose supply path are net losses despite cheaper DMA.
  - InstMatmult.ldweights=False is accepted but walrus still emits
    LDWEIGHTS (no effect).
  - HWDGE (sync/scalar dma_start) spreads over all 16 SDMA engines
    (~1MB in ~4us); but scalar-ring DMAs consume ACT-sequencer slots, so
    keep them off the relu-evac path during the chunk loop.
  - Ln and Exp live in different ACT table sets here (1.3us reload per
    switch); Sqrt+Relu share one set.
  - SBUF->SBUF DMA cannot relocate the partition dim (BIR: illegal
    partition step); the DRAM hop is required for the c-major transpose.
"""
import numpy as np
from contextlib import ExitStack

import concourse.bass as bass
import concourse.tile as tile
from concourse import bacc
from concourse import mybir
from concourse.bass_utils import run_bass_kernel_spmd

F32 = mybir.dt.float32
F16 = mybir.dt.float16
AX = mybir.AxisListType
OP = mybir.AluOpType
AF = mybir.ActivationFunctionType

EPS = 1e-8
B, C, P, S = 16, 76, 1024, 32
NCORES = 8
BL = B // NCORES
NQ = P // 128
NB = BL * NQ              # 16 q-groups across both batches
NCH = 32

GEO_W1_COLS = [67, 0, 70, 68, 1, 71, 69, 2, 72]

# A/B flag: drop the redundant LDWEIGHTS on the 2nd matmul of each
# same-weight pair (experimental walrus behavior).
SKIP_DUP_LDW = True


def build_program():
    nc = bacc.Bacc()

    feats_d = nc.dram_tensor("feats", [BL, 64, NQ, 4096], F16, kind="ExternalInput")
    geo_d = nc.dram_tensor("geo", [BL, 128, NQ, 3, 3, S], F16, kind="ExternalInput")
    norm_d = nc.dram_tensor("normp", [128, 3, BL, NQ], F32, kind="ExternalInput")
    w1c_d = nc.dram_tensor("w1c", [73, 128], F16, kind="ExternalInput")
    w2T_d = nc.dram_tensor("w2T", [128, 128], F16, kind="ExternalInput")
    b1_d = nc.dram_tensor("b1c", [128, 1], F32, kind="ExternalInput")
    b2_d = nc.dram_tensor("b2c", [128, 1], F32, kind="ExternalInput")
    geoT_d = nc.dram_tensor("geot", [BL, NQ, 9, 128, S], F16, kind="ExternalInput")
    outp_d = nc.dram_tensor("outp", [BL, 128, P], F16, kind="ExternalOutput")
    outa_d = nc.dram_tensor("outa", [128, 3, BL, NQ], F32, kind="ExternalOutput")

    with tile.TileContext(nc) as tc, ExitStack() as ctx:
        cpool = ctx.enter_context(tc.tile_pool(name="const", bufs=1))
        geo_pool = ctx.enter_context(tc.tile_pool(name="geo", bufs=2))
        rpool = ctx.enter_context(tc.tile_pool(name="rphase", bufs=1))
        al_pool = ctx.enter_context(tc.tile_pool(name="aligned", bufs=1))
        tmp_pool = ctx.enter_context(tc.tile_pool(name="rtmp", bufs=2))
        xt_pool = ctx.enter_context(tc.tile_pool(name="xt", bufs=1))
        h1_pool = ctx.enter_context(tc.tile_pool(name="h1", bufs=3))
        pb_pool = ctx.enter_context(tc.tile_pool(name="pooled", bufs=1))
        ps1_pool = ctx.enter_context(tc.tile_pool(name="ps1", bufs=2, space="PSUM"))
        ps2_pool = ctx.enter_context(tc.tile_pool(name="ps2", bufs=2, space="PSUM"))

        # ---- constants on sync HWDGE ----
        norm_pt = cpool.tile([128, 3, NB], F32)
        nc.sync.dma_start(out=norm_pt[:], in_=norm_d[:, :, :, :].rearrange("p x b q -> p x (b q)"))
        b1t = cpool.tile([128, 1], F32)
        nc.sync.dma_start(out=b1t[:], in_=b1_d[:, :])
        b2t = cpool.tile([128, 1], F32)
        nc.sync.dma_start(out=b2t[:], in_=b2_d[:, :])
        w1c = cpool.tile([73, 128], F16)
        nc.sync.dma_start(out=w1c[:], in_=w1c_d[:, :])
        w2T = cpool.tile([128, 128], F16)
        nc.sync.dma_start(out=w2T[:], in_=w2T_d[:, :])

        # dummy activation: hoists the one-time ACT_TABLE_LOAD off the
        # R-phase critical path (runs at t~2 while DMAs stream)
        tl_warm = cpool.tile([128, 1], F32)
        nc.scalar.activation(tl_warm[:], b1t[:], AF.Relu)

        geo_pt = {}
        xts = {}
        for _b in range(BL):
            xts[_b] = xt_pool.tile([73, NQ, 4096], F16, name=f"xt_{_b}")

        def emit_geo_load(b, ql=0, qh=NQ):
            if b in geo_pt:
                g = geo_pt[b]
            else:
                g = geo_pool.tile([128, NQ, 3, 3, S], F16, tag="geo")
                geo_pt[b] = g
            nc.scalar.dma_start(out=g[:, ql:qh], in_=geo_d[b, :, ql:qh])

        def emit_feats_load(b, q0, nq):
            nc.scalar.dma_start(
                out=xts[b][0:64, q0:q0 + nq].rearrange("c q f -> c (q f)"),
                in_=feats_d[b, :, q0:q0 + nq].rearrange("c q f -> c (q f)"))

        # ---------- R phase tiles ----------
        na = rpool.tile([128, 3, 2, NB], F32)
        sq2 = rpool.tile([128, 3, 2, NB], F32)
        ss2 = rpool.tile([128, 2, NB], F32)
        inv2 = rpool.tile([128, 2, NB], F32)
        u2 = rpool.tile([128, 3, 2, NB], F32)
        dot = rpool.tile([128, NB], F32)
        xraw = rpool.tile([128, 3, NB], F32)
        sqx = rpool.tile([128, 3, NB], F32)
        ssx = rpool.tile([128, NB], F32)
        nrmx = rpool.tile([128, NB], F32)
        invx = rpool.tile([128, NB], F32)
        x_u = rpool.tile([128, 3, NB], F32)
        yax = rpool.tile([128, 3, NB], F32)
        tmp3 = rpool.tile([128, 3, NB], F32)
        zero = rpool.tile([128, NB], F32)
        outa_sb = cpool.tile([128, 3, BL, NQ], F32)

        def emit_azi(b, ql=0, qh=NQ):
            """The only real reduction (mean over s) — on DVE, early."""
            B_ = slice(b * NQ + ql, b * NQ + qh)
            nc.vector.reduce_sum(
                out=na[:, :, 1, B_].transpose([0, 2, 1]),
                in_=geo_pt[b][:, ql:qh, :, 1, 1:S], axis=AX.X)

        def emit_rphase(b, eng, ql=0, qh=NQ):
            """R phase for one batch; everything on `eng` except sqrt (ACT).
            Unitization uses tensor divide instead of DVE reciprocal, so b1
            runs purely on gpsimd+ACT without touching the DVE queue after
            emit_azi."""
            B_ = slice(b * NQ + ql, b * NQ + qh)
            nq_ = qh - ql
            if b == 0 and ql == 0:
                eng.memset(zero[:], 0.0)
            eng.tensor_copy(out=na[:, :, 0, B_], in_=norm_pt[:, :, B_])
            eng.tensor_scalar_mul(out=na[:, :, 1, B_], in0=na[:, :, 1, B_],
                                  scalar1=1.0 / 31.0)
            nab = na[:, :, :, B_]
            eng.tensor_tensor(out=sq2[:, :, :, B_], in0=nab, in1=nab, op=OP.mult)
            eng.tensor_tensor(out=ss2[:, :, B_], in0=sq2[:, 0, :, B_],
                              in1=sq2[:, 1, :, B_], op=OP.add)
            eng.tensor_tensor(out=ss2[:, :, B_], in0=ss2[:, :, B_],
                              in1=sq2[:, 2, :, B_], op=OP.add)
            # 1/||v|| = exp(-0.5*ln(||v||^2 + 1e-12)) — ACT only, no DVE
            nc.scalar.activation(inv2[:, :, B_], ss2[:, :, B_], AF.Sqrt)
            nc.vector.reciprocal(inv2[:, :, B_], inv2[:, :, B_])
            inv_b = inv2[:, :, B_].unsqueeze(1).broadcast_to([128, 3, 2, nq_])
            eng.tensor_tensor(out=u2[:, :, :, B_], in0=nab, in1=inv_b,
                              op=OP.mult)
            n_u = u2[:, :, 0, B_]
            a_u = u2[:, :, 1, B_]

            eng.tensor_tensor(out=tmp3[:, :, B_], in0=a_u, in1=n_u, op=OP.mult)
            eng.tensor_tensor(out=dot[:, B_], in0=tmp3[:, 0, B_],
                              in1=tmp3[:, 1, B_], op=OP.add)
            eng.tensor_tensor(out=dot[:, B_], in0=dot[:, B_],
                              in1=tmp3[:, 2, B_], op=OP.add)

            dot_b = dot[:, B_].unsqueeze(1).broadcast_to([128, 3, nq_])
            eng.tensor_tensor(out=xraw[:, :, B_], in0=dot_b, in1=n_u, op=OP.mult)
            eng.tensor_tensor(out=xraw[:, :, B_], in0=a_u, in1=xraw[:, :, B_],
                              op=OP.subtract)
            eng.tensor_tensor(out=sqx[:, :, B_], in0=xraw[:, :, B_],
                              in1=xraw[:, :, B_], op=OP.mult)
            eng.tensor_tensor(out=ssx[:, B_], in0=sqx[:, 0, B_],
                              in1=sqx[:, 1, B_], op=OP.add)
            eng.tensor_tensor(out=ssx[:, B_], in0=ssx[:, B_],
                              in1=sqx[:, 2, B_], op=OP.add)
            nc.scalar.activation(nrmx[:, B_], ssx[:, B_], AF.Sqrt)
            nc.vector.reciprocal(invx[:, B_], nrmx[:, B_])
            invx_b = invx[:, B_].unsqueeze(1).broadcast_to([128, 3, nq_])
            eng.tensor_tensor(out=x_u[:, :, B_], in0=xraw[:, :, B_], in1=invx_b,
                              op=OP.mult)

            for x_ in range(3):
                i1, i2 = (x_ + 1) % 3, (x_ + 2) % 3
                eng.tensor_tensor(out=yax[:, x_, B_], in0=n_u[:, i1, :],
                                  in1=x_u[:, i2, B_], op=OP.mult)
                eng.tensor_tensor(out=tmp3[:, x_, B_], in0=n_u[:, i2, :],
                                  in1=x_u[:, i1, B_], op=OP.mult)
            eng.tensor_tensor(out=yax[:, :, B_], in0=yax[:, :, B_],
                              in1=tmp3[:, :, B_], op=OP.subtract)

            eng.tensor_copy(out=outa_sb[:, :, b, ql:qh], in_=a_u)

        def emit_rotation(eng, b, q0, nq):
            """aligned geo (c' = 3i+t) for q in [q0, q0+nq) of batch b on `eng`.
            svec rows: i=0 -> |xraw| (nrmx), i=1 -> 0, i=2 -> dot.
            i emitted in order [2, 0, 1]: n_u is ready before x_u/yax."""
            qs = slice(b * NQ + q0, b * NQ + q0 + nq)
            qsl = slice(q0, q0 + nq)
            rrows = [x_u, yax, u2[:, :, 0, :]]
            svs = [nrmx, zero, dot]
            alq = al_pool.tile([128, nq, 3, 3, S], F16, name=f"alq_{b}_{q0}")
            for i in (2, 0, 1):
                out3 = alq[:, :, i]
                for j in range(3):
                    rb = rrows[i][:, j, qs].unsqueeze(2).unsqueeze(3) \
                        .broadcast_to([128, nq, 3, S])
                    src = geo_pt[b][:, qsl, j]
                    if j == 0:
                        eng.tensor_tensor(out=out3, in0=src, in1=rb, op=OP.mult)
                    else:
                        t = tmp_pool.tile([128, nq, 3, S], F32, tag=f"rtmp{nq}")
                        eng.tensor_tensor(out=t[:], in0=src, in1=rb, op=OP.mult)
                        eng.tensor_tensor(out=out3, in0=out3, in1=t[:], op=OP.add)
                dir_row = alq[:, :, i, 2, :]
                sv_b = svs[i][:, qs].unsqueeze(2).broadcast_to([128, nq, S])
                eng.tensor_tensor(out=dir_row, in0=sv_b, in1=dir_row,
                                  op=OP.subtract)
            return alq

        def emit_rt_write(b, alq, q0, nq, eng):
            with tc.high_priority():
                eng.dma_start(
                    out=geoT_d[b, q0:q0 + nq].rearrange("q c p s -> p q c s"),
                    in_=alq[:].rearrange("p q i f s -> p q (i f) s"),
                )

        def emit_rt_read(b, q0, nq, eng):
            with tc.high_priority():
                eng.dma_start(
                    out=xts[b][64:73, q0:q0 + nq],
                    in_=geoT_d[b, q0:q0 + nq].rearrange(
                        "q c p s -> c q (p s)", p=128, s=S),
                )

        # ================= startup emission =================
        # gpsimd queue order matters (strict FIFO): keep bulk-load emissions
        # that are not immediately needed BEHIND the rt round trips.
        emit_geo_load(0)
        emit_geo_load(1)
        emit_feats_load(0, 0, 2)
        emit_feats_load(0, 2, 2)
        emit_feats_load(0, 4, 4)
        emit_feats_load(1, 0, 4)
        emit_feats_load(1, 4, 4)

        emit_azi(0)
        emit_rphase(0, nc.vector)
        # b0 g0/g1 rotation on DVE (idle during startup)
        alq_00 = emit_rotation(nc.vector, 0, 0, 2)
        # q0's round trip in p-halves: chunk 0 only needs points 0-63, so
        # its gate lands one half-transfer earlier
        with tc.high_priority():
            nc.sync.dma_start(
                out=geoT_d[0, 0:1, :, 0:64, :].rearrange("q c p s -> p q c s"),
                in_=alq_00[0:64, 0:1].rearrange("p q i f s -> p q (i f) s"))
            nc.sync.dma_start(
                out=xts[0][64:73, 0:1].rearrange("c q (p s) -> c q p s", p=128, s=S)[:, :, 0:64],
                in_=geoT_d[0, 0:1, :, 0:64, :].rearrange("q c p s -> c q (p s)"))
            nc.sync.dma_start(
                out=geoT_d[0, 0:1, :, 64:128, :].rearrange("q c p s -> p q c s"),
                in_=alq_00[64:128, 0:1].rearrange("p q i f s -> p q (i f) s"))
            nc.sync.dma_start(
                out=xts[0][64:73, 0:1].rearrange("c q (p s) -> c q p s", p=128, s=S)[:, :, 64:128],
                in_=geoT_d[0, 0:1, :, 64:128, :].rearrange("q c p s -> c q (p s)"))
        with tc.high_priority():
            nc.scalar.dma_start(
                out=geoT_d[0, 1:2, :, 0:64, :].rearrange("q c p s -> p q c s"),
                in_=alq_00[0:64, 1:2].rearrange("p q i f s -> p q (i f) s"))
            nc.scalar.dma_start(
                out=xts[0][64:73, 1:2].rearrange("c q (p s) -> c q p s", p=128, s=S)[:, :, 0:64],
                in_=geoT_d[0, 1:2, :, 0:64, :].rearrange("q c p s -> c q (p s)"))
            nc.scalar.dma_start(
                out=geoT_d[0, 1:2, :, 64:128, :].rearrange("q c p s -> p q c s"),
                in_=alq_00[64:128, 1:2].rearrange("p q i f s -> p q (i f) s"))
            nc.scalar.dma_start(
                out=xts[0][64:73, 1:2].rearrange("c q (p s) -> c q p s", p=128, s=S)[:, :, 64:128],
                in_=geoT_d[0, 1:2, :, 64:128, :].rearrange("q c p s -> c q (p s)"))

        emit_azi(1)
        # b1 R phase: elementwise on gpsimd, sqrt on ACT, reciprocal on DVE
        emit_rphase(1, nc.gpsimd)

        alq_02 = emit_rotation(nc.vector, 0, 2, 2)
        emit_rt_write(0, alq_02, 2, 2, nc.sync)
        emit_rt_read(0, 2, 2, nc.sync)

        # PE warmup on rot-g0 output, extended by a second burst gated on
        # rot-g1 so HAM stays at 8/8 across the rt round-trip gap to chunk 0
        warm_ps0 = ps1_pool.tile([128, 1024], F32, tag="h1ps")
        warm_rhs = alq_00[:].rearrange("p q i f s -> p (q i f s)")
        for _ in range(18):
            nc.tensor.matmul(out=warm_ps0[:, 0:512], lhsT=w2T[:],
                             rhs=warm_rhs[:, 0:512], start=True, stop=True)


        # b0 rotation tail on gpsimd
        for (b, q0, nq) in ((0, 4, 2), (0, 6, 2)):
            alq = emit_rotation(nc.gpsimd, b, q0, nq)
            emit_rt_write(b, alq, q0, nq, nc.sync)
            emit_rt_read(b, q0, nq, nc.sync)
        # b1 rotation in nq=2 groups so each rt round trip starts ~9us
        # earlier than the half-batch version
        for q0 in (0, 2, 4, 6):
            alq = emit_rotation(nc.gpsimd, 1, q0, 2)
            emit_rt_write(1, alq, q0, 2, nc.sync)
            emit_rt_read(1, q0, 2, nc.sync)

        pooled = {}
        pooled_raw = {}
        for b in range(BL):
            pooled[b] = pb_pool.tile([128, P], F16, name=f"pooled_{b}")
            pooled_raw[b] = pb_pool.tile([128, P], F16, name=f"pooledr_{b}")

        # ================= chunk loop =================
        # Software-pipelined: PE stream is mm1(0), mm1(1), mm2(0), mm1(2), ...
        chunks = [(b, k) for b in range(BL) for k in range(NCH)]

        def emit_mm1(idx):
            b, k = chunks[idx]
            ql, j = k // 4, k % 4
            base = j * 1024
            xt = xts[b]
            h1ps = ps1_pool.tile([128, 1024], F32, tag="h1ps")
            m0 = nc.tensor.matmul(out=h1ps[:, 0:512], lhsT=w1c[:],
                                  rhs=xt[:, ql, base:base + 512],
                                  start=True, stop=True)
            m1 = nc.tensor.matmul(out=h1ps[:, 512:1024], lhsT=w1c[:],
                                  rhs=xt[:, ql, base + 512:base + 1024],
                                  start=True, stop=True)
            if SKIP_DUP_LDW:
                m1.ins.ldweights = False
            return h1ps

        h1ps_cur = emit_mm1(0)
        for idx, (b, k) in enumerate(chunks):
            h1sb = h1_pool.tile([128, 1024], F16, tag="h1sb")
            nc.scalar.activation(h1sb[:], h1ps_cur[:], AF.Relu, bias=b1t[:, 0:1])
            if idx + 1 < len(chunks):
                h1ps_cur = emit_mm1(idx + 1)
            h2ps = ps2_pool.tile([128, 1024], F32, tag="h2ps")
            m0 = nc.tensor.matmul(out=h2ps[:, 0:512], lhsT=w2T[:],
                                  rhs=h1sb[:, 0:512], start=True, stop=True)
            m1 = nc.tensor.matmul(out=h2ps[:, 512:1024], lhsT=w2T[:],
                                  rhs=h1sb[:, 512:1024], start=True, stop=True)
            if SKIP_DUP_LDW:
                m1.ins.ldweights = False
            po = k * 32
            nc.vector.reduce_max(
                out=pooled_raw[b][:, po:po + 32],
                in_=h2ps[:].rearrange("m (p s) -> m p s", s=S),
                axis=AX.X)
            last_seg = b == BL - 1 and k >= NCH - 8
            if last_seg and (k == NCH - 5 or k == NCH - 1):
                seg = slice(po + 32 - 128, po + 32)
                nc.scalar.activation(pooled[b][:, seg], pooled_raw[b][:, seg],
                                     AF.Relu, bias=b2t[:, 0:1])
                nc.sync.dma_start(out=outp_d[b, :, seg], in_=pooled[b][:, seg])
            elif not last_seg and k % 8 == 7:
                seg = slice(po + 32 - 256, po + 32)
                nc.scalar.activation(pooled[b][:, seg], pooled_raw[b][:, seg],
                                     AF.Relu, bias=b2t[:, 0:1])
                nc.sync.dma_start(out=outp_d[b, :, seg], in_=pooled[b][:, seg])

        nc.scalar.dma_start(out=outa_d[:, :, :, :], in_=outa_sb[:])

    nc.finalize()
    return nc


_CACHE = {}


def _get_program():
    if "nc" not in _CACHE:
        _CACHE["nc"] = build_program()
    return _CACHE["nc"]


def make_in_maps(input, normal, w1, b1, w2, b2):
    input = np.asarray(input, dtype=np.float32)
    normal = np.asarray(normal, dtype=np.float32)
    w1 = np.asarray(w1, dtype=np.float32)
    b1 = np.asarray(b1, dtype=np.float32)
    w2 = np.asarray(w2, dtype=np.float32)
    b2 = np.asarray(b2, dtype=np.float32)

    w1fT = w1[:, 3:67].T.astype(np.float16)
    w1gT = w1[:, GEO_W1_COLS].T.astype(np.float16)
    w1c = np.ascontiguousarray(np.concatenate([w1fT, w1gT], axis=0))
    w2T = np.ascontiguousarray(w2.T.astype(np.float16))
    b1c = np.ascontiguousarray(b1.reshape(128, 1))
    b2c = np.ascontiguousarray(b2.reshape(128, 1))

    in_maps = []
    for core in range(NCORES):
        b0 = core * BL
        inp = input[b0:b0 + BL]
        f = inp[:, 12:76].astype(np.float16)
        feats = np.ascontiguousarray(f.reshape(BL, 64, NQ, 4096))
        g = inp[:, 3:12].astype(np.float16)
        g = g.reshape(BL, 3, 3, NQ, 128, S).transpose(0, 4, 3, 2, 1, 5)
        geo = np.ascontiguousarray(g)
        # normp [128, 3, BL, NQ]
        normp = np.ascontiguousarray(
            normal[b0:b0 + BL].reshape(BL, NQ, 128, 3).transpose(2, 3, 0, 1))
        in_maps.append({
            "feats": feats, "geo": geo, "normp": normp,
            "w1c": w1c, "w2T": w2T, "b1c": b1c, "b2c": b2c,
            "geot": np.zeros((BL, NQ, 9, 128, S), np.float16),
        })
    return in_maps


def assemble_output(results):
    outs = []
    for r in results:
        outp = r["outp"].astype(np.float32)   # (BL,128,P)
        outa = r["outa"]                      # (128,3,BL,NQ)
        azi = outa.transpose(2, 1, 3, 0).reshape(BL, 3, P)
        outs.append(np.concatenate([azi, outp], axis=1))
    return np.concatenate(outs, axis=0)


def kernel(input, normal, w1, b1, w2, b2, _trace=False):
    nc = _get_program()
    in_maps = make_in_maps(input, normal, w1, b1, w2, b2)
    res = run_bass_kernel_spmd(nc, in_maps, core_ids=list(range(NCORES)), trace=_trace)
    out = assemble_output(res.results)
    if _trace:
        return out, res
    return out


# revision 38
# speedup vs baseline: 1.1787x; 1.1787x over previous
"""Trainium2 Bass kernel for nn_ConvLayer_13967233646751 (gnn_message_passing).

Design (~146-148us on 8 cores, rel err 3.0e-3; baseline was 161us):
  - data-parallel over batch B=16 across 8 cores (2 batches/core)
  - host packs fp16: feats (64ch), geo (9ch, [p,q,j,t,s]), w1 column-
    permuted so GEMM1 is a single 73-row matmul per 512 columns.
  - R phase: rotation matrices from normals + azimuth means; xyz sums as
    elementwise adds (no mid-chain DVE reductions), sqrt on ACT (shares a
    table set with relu -> exactly one ACT_TABLE_LOAD), reciprocal on DVE.
    b0's R phase + first two rotation groups on DVE (idle at startup);
    b1's R phase elementwise + remaining rotations on gpsimd, all
    front-loaded before the chunk loop.
  - aligned geo goes p-major -> c-major via a DRAM round trip: scatter
    write (64B descriptors) + contiguous read, both on the sync HWDGE
    ring (bulk feats/geo loads live on the scalar HWDGE ring; the gpsimd
    SWDGE queue does no DMA at all, avoiding its in-flight slot limit).
  - chunk loop (64 x 1024 cols): PE mm1 (73x128) software-pipelined one
    chunk ahead of mm2 (128x128), ACT relu+bias evacuation, DVE max-pool
    from PSUM, pooled relu + outp DMA per 256 columns.
  - 8 warmup matmuls gated on the first rotation output keep the HAM
    clock gate warm into chunk 0; a dummy relu at t~2 hoists the one-time
    ACT_TABLE_LOAD off the R-phase critical path.
  - ring discipline (the big win, -14us): all bulk loads emitted first on
    the scalar ring; g0's rt round trip split q0-on-sync / q1-on-scalar
    (parallel rings); b1 rotation in nq=2 groups; outa on scalar at the
    end.  Rule: a DMA that will WAIT on a semaphore must never sit on the
    scalar/ACT ring while relu evacs run — it blocks the ACT queue head.

Hard-won notes (measured this session):
  - tile_position / small-K split matmuls keep the PE at K=4/8 half
    clock for the whole run (HAM never warms) -> split-GEMM1 designs and
    the xbar-transpose supply path are net losses despite cheaper DMA.
  - InstMatmult.ldweights=False is accepted but walrus still emits
    LDWEIGHTS (no effect).
  - HWDGE (sync/scalar dma_start) spreads over all 16 SDMA engines
    (~1MB in ~4us); but scalar-ring DMAs consume ACT-sequencer slots, so
    keep them off the relu-evac path during the chunk loop.
  - Ln and Exp live in different ACT table sets here (1.3us reload per
    switch); Sqrt+Relu share one set.
  - SBUF->SBUF DMA cannot relocate the partition dim (BIR: illegal
    partition step); the DRAM hop is required for the c-major transpose.
"""
import numpy as np
from contextlib import ExitStack

import concourse.bass as bass
import concourse.tile as tile
from concourse import bacc
from concourse import mybir
from concourse.bass_utils import run_bass_kernel_spmd

F32 = mybir.dt.float32
F16 = mybir.dt.float16
AX = mybir.AxisListType
OP = mybir.AluOpType
AF = mybir.ActivationFunctionType

EPS = 1e-8
B, C, P, S = 16, 76, 1024, 32
NCORES = 8
BL = B // NCORES
NQ = P // 128
NB = BL * NQ              # 16 q-groups across both batches
NCH = 32

GEO_W1_COLS = [67, 0, 70, 68, 1, 71, 69, 2, 72]

# A/B flag: drop the redundant LDWEIGHTS on the 2nd matmul of each
# same-weight pair (experimental walrus behavior).
SKIP_DUP_LDW = True


def build_program():
    nc = bacc.Bacc()

    feats_d = nc.dram_tensor("feats", [BL, 64, NQ, 4096], F16, kind="ExternalInput")
    geo_d = nc.dram_tensor("geo", [BL, 128, NQ, 3, 3, S], F16, kind="ExternalInput")
    norm_d = nc.dram_tensor("normp", [128, 3, BL, NQ], F32, kind="ExternalInput")
    w1c_d = nc.dram_tensor("w1c", [73, 128], F16, kind="ExternalInput")
    w2T_d = nc.dram_tensor("w2T", [128, 128], F16, kind="ExternalInput")
    b1_d = nc.dram_tensor("b1c", [128, 1], F32, kind="ExternalInput")
    b2_d = nc.dram_tensor("b2c", [128, 1], F32, kind="ExternalInput")
    geoT_d = nc.dram_tensor("geot", [BL, NQ, 9, 128, S], F16, kind="ExternalInput")
    outp_d = nc.dram_tensor("outp", [BL, 128, P], F16, kind="ExternalOutput")
    outa_d = nc.dram_tensor("outa", [128, 3, BL, NQ], F32, kind="ExternalOutput")

    with tile.TileContext(nc) as tc, ExitStack() as ctx:
        cpool = ctx.enter_context(tc.tile_pool(name="const", bufs=1))
        geo_pool = ctx.enter_context(tc.tile_pool(name="geo", bufs=2))
        rpool = ctx.enter_context(tc.tile_pool(name="rphase", bufs=1))
        al_pool = ctx.enter_context(tc.tile_pool(name="aligned", bufs=1))
        tmp_pool = ctx.enter_context(tc.tile_pool(name="rtmp", bufs=2))
        xt_pool = ctx.enter_context(tc.tile_pool(name="xt", bufs=1))
        h1_pool = ctx.enter_context(tc.tile_pool(name="h1", bufs=3))
        pb_pool = ctx.enter_context(tc.tile_pool(name="pooled", bufs=1))
        ps1_pool = ctx.enter_context(tc.tile_pool(name="ps1", bufs=2, space="PSUM"))
        ps2_pool = ctx.enter_context(tc.tile_pool(name="ps2", bufs=2, space="PSUM"))

        # ---- constants on sync HWDGE ----
        norm_pt = cpool.tile([128, 3, NB], F32)
        nc.sync.dma_start(out=norm_pt[:], in_=norm_d[:, :, :, :].rearrange("p x b q -> p x (b q)"))
        b1t = cpool.tile([128, 1], F32)
        nc.sync.dma_start(out=b1t[:], in_=b1_d[:, :])
        b2t = cpool.tile([128, 1], F32)
        nc.sync.dma_start(out=b2t[:], in_=b2_d[:, :])
        w1c = cpool.tile([73, 128], F16)
        nc.sync.dma_start(out=w1c[:], in_=w1c_d[:, :])
        w2T = cpool.tile([128, 128], F16)
        nc.sync.dma_start(out=w2T[:], in_=w2T_d[:, :])

        # dummy activation: hoists the one-time ACT_TABLE_LOAD off the
        # R-phase critical path (runs at t~2 while DMAs stream)
        tl_warm = cpool.tile([128, 1], F32)
        nc.scalar.activation(tl_warm[:], b1t[:], AF.Relu)

        geo_pt = {}
        xts = {}
        for _b in range(BL):
            xts[_b] = xt_pool.tile([73, NQ, 4096], F16, name=f"xt_{_b}")

        def emit_geo_load(b, ql=0, qh=NQ):
            if b in geo_pt:
                g = geo_pt[b]
            else:
                g = geo_pool.tile([128, NQ, 3, 3, S], F16, tag="geo")
                geo_pt[b] = g
            nc.scalar.dma_start(out=g[:, ql:qh], in_=geo_d[b, :, ql:qh])

        def emit_feats_load(b, q0, nq):
            nc.scalar.dma_start(
                out=xts[b][0:64, q0:q0 + nq].rearrange("c q f -> c (q f)"),
                in_=feats_d[b, :, q0:q0 + nq].rearrange("c q f -> c (q f)"))

        # ---------- R phase tiles ----------
        na = rpool.tile([128, 3, 2, NB], F32)
        sq2 = rpool.tile([128, 3, 2, NB], F32)
        ss2 = rpool.tile([128, 2, NB], F32)
        inv2 = rpool.tile([128, 2, NB], F32)
        u2 = rpool.tile([128, 3, 2, NB], F32)
        dot = rpool.tile([128, NB], F32)
        xraw = rpool.tile([128, 3, NB], F32)
        sqx = rpool.tile([128, 3, NB], F32)
        ssx = rpool.tile([128, NB], F32)
        nrmx = rpool.tile([128, NB], F32)
        invx = rpool.tile([128, NB], F32)
        x_u = rpool.tile([128, 3, NB], F32)
        yax = rpool.tile([128, 3, NB], F32)
        tmp3 = rpool.tile([128, 3, NB], F32)
        zero = rpool.tile([128, NB], F32)
        outa_sb = cpool.tile([128, 3, BL, NQ], F32)

        def emit_azi(b, ql=0, qh=NQ):
            """The only real reduction (mean over s) — on DVE, early."""
            B_ = slice(b * NQ + ql, b * NQ + qh)
            nc.vector.reduce_sum(
                out=na[:, :, 1, B_].transpose([0, 2, 1]),
                in_=geo_pt[b][:, ql:qh, :, 1, 1:S], axis=AX.X)

        def emit_rphase(b, eng, ql=0, qh=NQ):
            """R phase for one batch; everything on `eng` except sqrt (ACT).
            Unitization uses tensor divide instead of DVE reciprocal, so b1
            runs purely on gpsimd+ACT without touching the DVE queue after
            emit_azi."""
            B_ = slice(b * NQ + ql, b * NQ + qh)
            nq_ = qh - ql
            if b == 0 and ql == 0:
                eng.memset(zero[:], 0.0)
            eng.tensor_copy(out=na[:, :, 0, B_], in_=norm_pt[:, :, B_])
            eng.tensor_scalar_mul(out=na[:, :, 1, B_], in0=na[:, :, 1, B_],
                                  scalar1=1.0 / 31.0)
            nab = na[:, :, :, B_]
            eng.tensor_tensor(out=sq2[:, :, :, B_], in0=nab, in1=nab, op=OP.mult)
            eng.tensor_tensor(out=ss2[:, :, B_], in0=sq2[:, 0, :, B_],
                              in1=sq2[:, 1, :, B_], op=OP.add)
            eng.tensor_tensor(out=ss2[:, :, B_], in0=ss2[:, :, B_],
                              in1=sq2[:, 2, :, B_], op=OP.add)
            # 1/||v|| = exp(-0.5*ln(||v||^2 + 1e-12)) — ACT only, no DVE
            nc.scalar.activation(inv2[:, :, B_], ss2[:, :, B_], AF.Sqrt)
            eng.tensor_scalar_add(out=inv2[:, :, B_], in0=inv2[:, :, B_],
                                  scalar1=EPS)
            nc.vector.reciprocal(inv2[:, :, B_], inv2[:, :, B_])
            inv_b = inv2[:, :, B_].unsqueeze(1).broadcast_to([128, 3, 2, nq_])
            eng.tensor_tensor(out=u2[:, :, :, B_], in0=nab, in1=inv_b,
                              op=OP.mult)
            n_u = u2[:, :, 0, B_]
            a_u = u2[:, :, 1, B_]

            eng.tensor_tensor(out=tmp3[:, :, B_], in0=a_u, in1=n_u, op=OP.mult)
            eng.tensor_tensor(out=dot[:, B_], in0=tmp3[:, 0, B_],
                              in1=tmp3[:, 1, B_], op=OP.add)
            eng.tensor_tensor(out=dot[:, B_], in0=dot[:, B_],
                              in1=tmp3[:, 2, B_], op=OP.add)

            dot_b = dot[:, B_].unsqueeze(1).broadcast_to([128, 3, nq_])
            eng.tensor_tensor(out=xraw[:, :, B_], in0=dot_b, in1=n_u, op=OP.mult)
            eng.tensor_tensor(out=xraw[:, :, B_], in0=a_u, in1=xraw[:, :, B_],
                              op=OP.subtract)
            eng.tensor_tensor(out=sqx[:, :, B_], in0=xraw[:, :, B_],
                              in1=xraw[:, :, B_], op=OP.mult)
            eng.tensor_tensor(out=ssx[:, B_], in0=sqx[:, 0, B_],
                              in1=sqx[:, 1, B_], op=OP.add)
            eng.tensor_tensor(out=ssx[:, B_], in0=ssx[:, B_],
                              in1=sqx[:, 2, B_], op=OP.add)
            nc.scalar.activation(nrmx[:, B_], ssx[:, B_], AF.Sqrt)
            eng.tensor_scalar_add(out=nrmx[:, B_], in0=nrmx[:, B_], scalar1=EPS)
            nc.vector.reciprocal(invx[:, B_], nrmx[:, B_])
            invx_b = invx[:, B_].unsqueeze(1).broadcast_to([128, 3, nq_])
            eng.tensor_tensor(out=x_u[:, :, B_], in0=xraw[:, :, B_], in1=invx_b,
                              op=OP.mult)

            for x_ in range(3):
                i1, i2 = (x_ + 1) % 3, (x_ + 2) % 3
                eng.tensor_tensor(out=yax[:, x_, B_], in0=n_u[:, i1, :],
                                  in1=x_u[:, i2, B_], op=OP.mult)
                eng.tensor_tensor(out=tmp3[:, x_, B_], in0=n_u[:, i2, :],
                                  in1=x_u[:, i1, B_], op=OP.mult)
            eng.tensor_tensor(out=yax[:, :, B_], in0=yax[:, :, B_],
                              in1=tmp3[:, :, B_], op=OP.subtract)

            eng.tensor_copy(out=outa_sb[:, :, b, ql:qh], in_=a_u)

        def emit_rotation(eng, b, q0, nq):
            """aligned geo (c' = 3i+t) for q in [q0, q0+nq) of batch b on `eng`.
            svec rows: i=0 -> |xraw| (nrmx), i=1 -> 0, i=2 -> dot.
            i emitted in order [2, 0, 1]: n_u is ready before x_u/yax."""
            qs = slice(b * NQ + q0, b * NQ + q0 + nq)
            qsl = slice(q0, q0 + nq)
            rrows = [x_u, yax, u2[:, :, 0, :]]
            svs = [nrmx, zero, dot]
            alq = al_pool.tile([128, nq, 3, 3, S], F16, name=f"alq_{b}_{q0}")
            for i in (2, 0, 1):
                out3 = alq[:, :, i]
                for j in range(3):
                    rb = rrows[i][:, j, qs].unsqueeze(2).unsqueeze(3) \
                        .broadcast_to([128, nq, 3, S])
                    src = geo_pt[b][:, qsl, j]
                    if j == 0:
                        eng.tensor_tensor(out=out3, in0=src, in1=rb, op=OP.mult)
                    else:
                        t = tmp_pool.tile([128, nq, 3, S], F32, tag=f"rtmp{nq}")
                        eng.tensor_tensor(out=t[:], in0=src, in1=rb, op=OP.mult)
                        eng.tensor_tensor(out=out3, in0=out3, in1=t[:], op=OP.add)
                dir_row = alq[:, :, i, 2, :]
                sv_b = svs[i][:, qs].unsqueeze(2).broadcast_to([128, nq, S])
                eng.tensor_tensor(out=dir_row, in0=sv_b, in1=dir_row,
                                  op=OP.subtract)
            return alq

        def emit_rt_write(b, alq, q0, nq, eng):
            with tc.high_priority():
                eng.dma_start(
                    out=geoT_d[b, q0:q0 + nq].rearrange("q c p s -> p q c s"),
                    in_=alq[:].rearrange("p q i f s -> p q (i f) s"),
                )

        def emit_rt_read(b, q0, nq, eng):
            with tc.high_priority():
                eng.dma_start(
                    out=xts[b][64:73, q0:q0 + nq],
                    in_=geoT_d[b, q0:q0 + nq].rearrange(
                        "q c p s -> c q (p s)", p=128, s=S),
                )

        # ================= startup emission =================
        # gpsimd queue order matters (strict FIFO): keep bulk-load emissions
        # that are not immediately needed BEHIND the rt round trips.
        emit_geo_load(0)
        emit_geo_load(1)
        emit_feats_load(0, 0, 2)
        emit_feats_load(0, 2, 2)
        emit_feats_load(0, 4, 4)
        emit_feats_load(1, 0, 4)
        emit_feats_load(1, 4, 4)

        emit_azi(0)
        emit_rphase(0, nc.vector)
        emit_azi(1)
        # b0 g0/g1 rotation on DVE (idle during startup)
        alq_00 = emit_rotation(nc.vector, 0, 0, 2)
        # q0's round trip in p-halves: chunk 0 only needs points 0-63, so
        # its gate lands one half-transfer earlier
        with tc.high_priority():
            nc.sync.dma_start(
                out=geoT_d[0, 0:1, :, 0:64, :].rearrange("q c p s -> p q c s"),
                in_=alq_00[0:64, 0:1].rearrange("p q i f s -> p q (i f) s"))
            nc.sync.dma_start(
                out=xts[0][64:73, 0:1].rearrange("c q (p s) -> c q p s", p=128, s=S)[:, :, 0:64],
                in_=geoT_d[0, 0:1, :, 0:64, :].rearrange("q c p s -> c q (p s)"))
            nc.sync.dma_start(
                out=geoT_d[0, 0:1, :, 64:128, :].rearrange("q c p s -> p q c s"),
                in_=alq_00[64:128, 0:1].rearrange("p q i f s -> p q (i f) s"))
            nc.sync.dma_start(
                out=xts[0][64:73, 0:1].rearrange("c q (p s) -> c q p s", p=128, s=S)[:, :, 64:128],
                in_=geoT_d[0, 0:1, :, 64:128, :].rearrange("q c p s -> c q (p s)"))
        with tc.high_priority():
            nc.scalar.dma_start(
                out=geoT_d[0, 1:2, :, 0:64, :].rearrange("q c p s -> p q c s"),
                in_=alq_00[0:64, 1:2].rearrange("p q i f s -> p q (i f) s"))
            nc.scalar.dma_start(
                out=xts[0][64:73, 1:2].rearrange("c q (p s) -> c q p s", p=128, s=S)[:, :, 0:64],
                in_=geoT_d[0, 1:2, :, 0:64, :].rearrange("q c p s -> c q (p s)"))
            nc.scalar.dma_start(
                out=geoT_d[0, 1:2, :, 64:128, :].rearrange("q c p s -> p q c s"),
                in_=alq_00[64:128, 1:2].rearrange("p q i f s -> p q (i f) s"))
            nc.scalar.dma_start(
                out=xts[0][64:73, 1:2].rearrange("c q (p s) -> c q p s", p=128, s=S)[:, :, 64:128],
                in_=geoT_d[0, 1:2, :, 64:128, :].rearrange("q c p s -> c q (p s)"))

        # b1 R phase: elementwise on gpsimd, sqrt on ACT, reciprocal on DVE
        emit_rphase(1, nc.gpsimd)

        alq_02 = emit_rotation(nc.vector, 0, 2, 2)
        emit_rt_write(0, alq_02, 2, 2, nc.sync)
        emit_rt_read(0, 2, 2, nc.sync)

        # PE warmup on rot-g0 output, extended by a second burst gated on
        # rot-g1 so HAM stays at 8/8 across the rt round-trip gap to chunk 0
        warm_ps0 = ps1_pool.tile([128, 1024], F32, tag="h1ps")
        warm_rhs = alq_00[:].rearrange("p q i f s -> p (q i f s)")
        for _ in range(18):
            nc.tensor.matmul(out=warm_ps0[:, 0:512], lhsT=w2T[:],
                             rhs=warm_rhs[:, 0:512], start=True, stop=True)


        # b0 rotation tail on gpsimd
        for (b, q0, nq) in ((0, 4, 2), (0, 6, 2)):
            alq = emit_rotation(nc.gpsimd, b, q0, nq)
            emit_rt_write(b, alq, q0, nq, nc.sync)
            emit_rt_read(b, q0, nq, nc.sync)
        # b1 rotation in nq=2 groups so each rt round trip starts ~9us
        # earlier than the half-batch version
        for q0 in (0, 2, 4, 6):
            alq = emit_rotation(nc.gpsimd, 1, q0, 2)
            emit_rt_write(1, alq, q0, 2, nc.sync)
            emit_rt_read(1, q0, 2, nc.sync)

        pooled = {}
        pooled_raw = {}
        for b in range(BL):
            pooled[b] = pb_pool.tile([128, P], F16, name=f"pooled_{b}")
            pooled_raw[b] = pb_pool.tile([128, P], F16, name=f"pooledr_{b}")

        # ================= chunk loop =================
        # Software-pipelined: PE stream is mm1(0), mm1(1), mm2(0), mm1(2), ...
        chunks = [(b, k) for b in range(BL) for k in range(NCH)]

        def emit_mm1(idx):
            b, k = chunks[idx]
            ql, j = k // 4, k % 4
            base = j * 1024
            xt = xts[b]
            h1ps = ps1_pool.tile([128, 1024], F32, tag="h1ps")
            m0 = nc.tensor.matmul(out=h1ps[:, 0:512], lhsT=w1c[:],
                                  rhs=xt[:, ql, base:base + 512],
                                  start=True, stop=True)
            m1 = nc.tensor.matmul(out=h1ps[:, 512:1024], lhsT=w1c[:],
                                  rhs=xt[:, ql, base + 512:base + 1024],
                                  start=True, stop=True)
            if SKIP_DUP_LDW:
                m1.ins.ldweights = False
            return h1ps

        h1ps_cur = emit_mm1(0)
        for idx, (b, k) in enumerate(chunks):
            h1sb = h1_pool.tile([128, 1024], F16, tag="h1sb")
            nc.scalar.activation(h1sb[:], h1ps_cur[:], AF.Relu, bias=b1t[:, 0:1])
            if idx + 1 < len(chunks):
                h1ps_cur = emit_mm1(idx + 1)
            h2ps = ps2_pool.tile([128, 1024], F32, tag="h2ps")
            m0 = nc.tensor.matmul(out=h2ps[:, 0:512], lhsT=w2T[:],
                                  rhs=h1sb[:, 0:512], start=True, stop=True)
            m1 = nc.tensor.matmul(out=h2ps[:, 512:1024], lhsT=w2T[:],
                                  rhs=h1sb[:, 512:1024], start=True, stop=True)
            if SKIP_DUP_LDW:
                m1.ins.ldweights = False
            po = k * 32
            nc.vector.reduce_max(
                out=pooled_raw[b][:, po:po + 32],
                in_=h2ps[:].rearrange("m (p s) -> m p s", s=S),
                axis=AX.X)
            last_seg = b == BL - 1 and k >= NCH - 8
            if last_seg and (k == NCH - 5 or k == NCH - 1):
                seg = slice(po + 32 - 128, po + 32)
                nc.scalar.activation(pooled[b][:, seg], pooled_raw[b][:, seg],
                                     AF.Relu, bias=b2t[:, 0:1])
                nc.sync.dma_start(out=outp_d[b, :, seg], in_=pooled[b][:, seg])
            elif not last_seg and k % 8 == 7:
                seg = slice(po + 32 - 256, po + 32)
                nc.scalar.activation(pooled[b][:, seg], pooled_raw[b][:, seg],
                                     AF.Relu, bias=b2t[:, 0:1])
                nc.sync.dma_start(out=outp_d[b, :, seg], in_=pooled[b][:, seg])

        nc.scalar.dma_start(out=outa_d[:, :, :, :], in_=outa_sb[:])

    nc.finalize()
    return nc


_CACHE = {}


def _get_program():
    if "nc" not in _CACHE:
        _CACHE["nc"] = build_program()
    return _CACHE["nc"]


def make_in_maps(input, normal, w1, b1, w2, b2):
    input = np.asarray(input, dtype=np.float32)
    normal = np.asarray(normal, dtype=np.float32)
    w1 = np.asarray(w1, dtype=np.float32)
    b1 = np.asarray(b1, dtype=np.float32)
    w2 = np.asarray(w2, dtype=np.float32)
    b2 = np.asarray(b2, dtype=np.float32)

    w1fT = w1[:, 3:67].T.astype(np.float16)
    w1gT = w1[:, GEO_W1_COLS].T.astype(np.float16)
    w1c = np.ascontiguousarray(np.concatenate([w1fT, w1gT], axis=0))
    w2T = np.ascontiguousarray(w2.T.astype(np.float16))
    b1c = np.ascontiguousarray(b1.reshape(128, 1))
    b2c = np.ascontiguousarray(b2.reshape(128, 1))

    in_maps = []
    for core in range(NCORES):
        b0 = core * BL
        inp = input[b0:b0 + BL]
        f = inp[:, 12:76].astype(np.float16)
        feats = np.ascontiguousarray(f.reshape(BL, 64, NQ, 4096))
        g = inp[:, 3:12].astype(np.float16)
        g = g.reshape(BL, 3, 3, NQ, 128, S).transpose(0, 4, 3, 2, 1, 5)
        geo = np.ascontiguousarray(g)
        # normp [128, 3, BL, NQ]
        normp = np.ascontiguousarray(
            normal[b0:b0 + BL].reshape(BL, NQ, 128, 3).transpose(2, 3, 0, 1))
        in_maps.append({
            "feats": feats, "geo": geo, "normp": normp,
            "w1c": w1c, "w2T": w2T, "b1c": b1c, "b2c": b2c,
            "geot": np.zeros((BL, NQ, 9, 128, S), np.float16),
        })
    return in_maps


def assemble_output(results):
    outs = []
    for r in results:
        outp = r["outp"].astype(np.float32)   # (BL,128,P)
        outa = r["outa"]                      # (128,3,BL,NQ)
        azi = outa.transpose(2, 1, 3, 0).reshape(BL, 3, P)
        outs.append(np.concatenate([azi, outp], axis=1))
    return np.concatenate(outs, axis=0)


def kernel(input, normal, w1, b1, w2, b2, _trace=False):
    nc = _get_program()
    in_maps = make_in_maps(input, normal, w1, b1, w2, b2)
    res = run_bass_kernel_spmd(nc, in_maps, core_ids=list(range(NCORES)), trace=_trace)
    out = assemble_output(res.results)
    if _trace:
        return out, res
    return out


# revision 40
# speedup vs baseline: 1.1907x; 1.0102x over previous
"""Trainium2 Bass kernel for nn_ConvLayer_13967233646751 (gnn_message_passing).

Design (~146-148us on 8 cores, rel err 3.0e-3; baseline was 161us):
  - data-parallel over batch B=16 across 8 cores (2 batches/core)
  - host packs fp16: feats (64ch), geo (9ch, [p,q,j,t,s]), w1 column-
    permuted so GEMM1 is a single 73-row matmul per 512 columns.
  - R phase: rotation matrices from normals + azimuth means; xyz sums as
    elementwise adds (no mid-chain DVE reductions), sqrt on ACT (shares a
    table set with relu -> exactly one ACT_TABLE_LOAD), reciprocal on DVE.
    b0's R phase + first two rotation groups on DVE (idle at startup);
    b1's R phase elementwise + remaining rotations on gpsimd, all
    front-loaded before the chunk loop.
  - aligned geo goes p-major -> c-major via a DRAM round trip: scatter
    write (64B descriptors) + contiguous read, both on the sync HWDGE
    ring (bulk feats/geo loads live on the scalar HWDGE ring; the gpsimd
    SWDGE queue does no DMA at all, avoiding its in-flight slot limit).
  - chunk loop (64 x 1024 cols): PE mm1 (73x128) software-pipelined one
    chunk ahead of mm2 (128x128), ACT relu+bias evacuation, DVE max-pool
    from PSUM, pooled relu + outp DMA per 256 columns.
  - 8 warmup matmuls gated on the first rotation output keep the HAM
    clock gate warm into chunk 0; a dummy relu at t~2 hoists the one-time
    ACT_TABLE_LOAD off the R-phase critical path.
  - ring discipline (the big win, -14us): all bulk loads emitted first on
    the scalar ring; g0's rt round trip split q0-on-sync / q1-on-scalar
    (parallel rings); b1 rotation in nq=2 groups; outa on scalar at the
    end.  Rule: a DMA that will WAIT on a semaphore must never sit on the
    scalar/ACT ring while relu evacs run — it blocks the ACT queue head.

Hard-won notes (measured this session):
  - tile_position / small-K split matmuls keep the PE at K=4/8 half
    clock for the whole run (HAM never warms) -> split-GEMM1 designs and
    the xbar-transpose supply path are net losses despite cheaper DMA.
  - InstMatmult.ldweights=False is accepted but walrus still emits
    LDWEIGHTS (no effect).
  - HWDGE (sync/scalar dma_start) spreads over all 16 SDMA engines
    (~1MB in ~4us); but scalar-ring DMAs consume ACT-sequencer slots, so
    keep them off the relu-evac path during the chunk loop.
  - Ln and Exp live in different ACT table sets here (1.3us reload per
    switch); Sqrt+Relu share one set.
  - SBUF->SBUF DMA cannot relocate the partition dim (BIR: illegal
    partition step); the DRAM hop is required for the c-major transpose.
"""
import numpy as np
from contextlib import ExitStack

import concourse.bass as bass
import concourse.tile as tile
from concourse import bacc
from concourse import mybir
from concourse.bass_utils import run_bass_kernel_spmd

F32 = mybir.dt.float32
F16 = mybir.dt.float16
AX = mybir.AxisListType
OP = mybir.AluOpType
AF = mybir.ActivationFunctionType

EPS = 1e-8
B, C, P, S = 16, 76, 1024, 32
NCORES = 8
BL = B // NCORES
NQ = P // 128
NB = BL * NQ              # 16 q-groups across both batches
NCH = 32

GEO_W1_COLS = [67, 0, 70, 68, 1, 71, 69, 2, 72]

# A/B flag: drop the redundant LDWEIGHTS on the 2nd matmul of each
# same-weight pair (experimental walrus behavior).
SKIP_DUP_LDW = True


def build_program():
    nc = bacc.Bacc()

    feats_d = nc.dram_tensor("feats", [BL, 64, NQ, 4096], F16, kind="ExternalInput")
    geo_d = nc.dram_tensor("geo", [BL, 128, NQ, 3, 3, S], F16, kind="ExternalInput")
    norm_d = nc.dram_tensor("normp", [128, 3, BL, NQ], F32, kind="ExternalInput")
    w1c_d = nc.dram_tensor("w1c", [73, 128], F16, kind="ExternalInput")
    w2T_d = nc.dram_tensor("w2T", [128, 128], F16, kind="ExternalInput")
    b1_d = nc.dram_tensor("b1c", [128, 1], F32, kind="ExternalInput")
    b2_d = nc.dram_tensor("b2c", [128, 1], F32, kind="ExternalInput")
    geoT_d = nc.dram_tensor("geot", [BL, NQ, 9, 128, S], F16, kind="ExternalInput")
    outp_d = nc.dram_tensor("outp", [BL, 128, P], F16, kind="ExternalOutput")
    outa_d = nc.dram_tensor("outa", [128, 3, BL, NQ], F32, kind="ExternalOutput")

    with tile.TileContext(nc) as tc, ExitStack() as ctx:
        cpool = ctx.enter_context(tc.tile_pool(name="const", bufs=1))
        geo_pool = ctx.enter_context(tc.tile_pool(name="geo", bufs=2))
        rpool = ctx.enter_context(tc.tile_pool(name="rphase", bufs=1))
        al_pool = ctx.enter_context(tc.tile_pool(name="aligned", bufs=1))
        tmp_pool = ctx.enter_context(tc.tile_pool(name="rtmp", bufs=2))
        xt_pool = ctx.enter_context(tc.tile_pool(name="xt", bufs=1))
        h1_pool = ctx.enter_context(tc.tile_pool(name="h1", bufs=3))
        pb_pool = ctx.enter_context(tc.tile_pool(name="pooled", bufs=1))
        ps1_pool = ctx.enter_context(tc.tile_pool(name="ps1", bufs=2, space="PSUM"))
        ps2_pool = ctx.enter_context(tc.tile_pool(name="ps2", bufs=2, space="PSUM"))

        # ---- constants on sync HWDGE ----
        norm_pt = cpool.tile([128, 3, NB], F32)
        nc.sync.dma_start(out=norm_pt[:], in_=norm_d[:, :, :, :].rearrange("p x b q -> p x (b q)"))
        b1t = cpool.tile([128, 1], F32)
        nc.sync.dma_start(out=b1t[:], in_=b1_d[:, :])
        b2t = cpool.tile([128, 1], F32)
        nc.sync.dma_start(out=b2t[:], in_=b2_d[:, :])
        w1c = cpool.tile([73, 128], F16)
        nc.sync.dma_start(out=w1c[:], in_=w1c_d[:, :])
        w2T = cpool.tile([128, 128], F16)
        nc.sync.dma_start(out=w2T[:], in_=w2T_d[:, :])

        # dummy activation: hoists the one-time ACT_TABLE_LOAD off the
        # R-phase critical path (runs at t~2 while DMAs stream)
        tl_warm = cpool.tile([128, 1], F32)
        nc.scalar.activation(tl_warm[:], b1t[:], AF.Relu)

        geo_pt = {}
        xts = {}
        for _b in range(BL):
            xts[_b] = xt_pool.tile([73, NQ, 4096], F16, name=f"xt_{_b}")

        def emit_geo_load(b, ql=0, qh=NQ):
            if b in geo_pt:
                g = geo_pt[b]
            else:
                g = geo_pool.tile([128, NQ, 3, 3, S], F16, tag="geo")
                geo_pt[b] = g
            nc.scalar.dma_start(out=g[:, ql:qh], in_=geo_d[b, :, ql:qh])

        def emit_feats_load(b, q0, nq):
            nc.scalar.dma_start(
                out=xts[b][0:64, q0:q0 + nq].rearrange("c q f -> c (q f)"),
                in_=feats_d[b, :, q0:q0 + nq].rearrange("c q f -> c (q f)"))

        # ---------- R phase tiles ----------
        na = rpool.tile([128, 3, 2, NB], F32)
        sq2 = rpool.tile([128, 3, 2, NB], F32)
        ss2 = rpool.tile([128, 2, NB], F32)
        inv2 = rpool.tile([128, 2, NB], F32)
        u2 = rpool.tile([128, 3, 2, NB], F32)
        dot = rpool.tile([128, NB], F32)
        xraw = rpool.tile([128, 3, NB], F32)
        sqx = rpool.tile([128, 3, NB], F32)
        ssx = rpool.tile([128, NB], F32)
        nrmx = rpool.tile([128, NB], F32)
        invx = rpool.tile([128, NB], F32)
        x_u = rpool.tile([128, 3, NB], F32)
        yax = rpool.tile([128, 3, NB], F32)
        tmp3 = rpool.tile([128, 3, NB], F32)
        zero = rpool.tile([128, NB], F32)
        outa_sb = cpool.tile([128, 3, BL, NQ], F32)

        def emit_azi(b, ql=0, qh=NQ):
            """The only real reduction (mean over s) — on DVE, early."""
            B_ = slice(b * NQ + ql, b * NQ + qh)
            nc.vector.reduce_sum(
                out=na[:, :, 1, B_].transpose([0, 2, 1]),
                in_=geo_pt[b][:, ql:qh, :, 1, 1:S], axis=AX.X)

        def emit_rphase(b, eng, ql=0, qh=NQ):
            """R phase for one batch; everything on `eng` except sqrt (ACT).
            Unitization uses tensor divide instead of DVE reciprocal, so b1
            runs purely on gpsimd+ACT without touching the DVE queue after
            emit_azi."""
            B_ = slice(b * NQ + ql, b * NQ + qh)
            nq_ = qh - ql
            if b == 0 and ql == 0:
                eng.memset(zero[:], 0.0)
            eng.tensor_copy(out=na[:, :, 0, B_], in_=norm_pt[:, :, B_])
            eng.tensor_scalar_mul(out=na[:, :, 1, B_], in0=na[:, :, 1, B_],
                                  scalar1=1.0 / 31.0)
            nab = na[:, :, :, B_]
            eng.tensor_tensor(out=sq2[:, :, :, B_], in0=nab, in1=nab, op=OP.mult)
            eng.tensor_tensor(out=ss2[:, :, B_], in0=sq2[:, 0, :, B_],
                              in1=sq2[:, 1, :, B_], op=OP.add)
            eng.tensor_tensor(out=ss2[:, :, B_], in0=ss2[:, :, B_],
                              in1=sq2[:, 2, :, B_], op=OP.add)
            # 1/||v|| = exp(-0.5*ln(||v||^2 + 1e-12)) — ACT only, no DVE
            nc.scalar.activation(inv2[:, :, B_], ss2[:, :, B_], AF.Sqrt)
            eng.tensor_scalar_add(out=inv2[:, :, B_], in0=inv2[:, :, B_],
                                  scalar1=EPS)
            nc.vector.reciprocal(inv2[:, :, B_], inv2[:, :, B_])
            inv_b = inv2[:, :, B_].unsqueeze(1).broadcast_to([128, 3, 2, nq_])
            eng.tensor_tensor(out=u2[:, :, :, B_], in0=nab, in1=inv_b,
                              op=OP.mult)
            n_u = u2[:, :, 0, B_]
            a_u = u2[:, :, 1, B_]

            eng.tensor_tensor(out=tmp3[:, :, B_], in0=a_u, in1=n_u, op=OP.mult)
            eng.tensor_tensor(out=dot[:, B_], in0=tmp3[:, 0, B_],
                              in1=tmp3[:, 1, B_], op=OP.add)
            eng.tensor_tensor(out=dot[:, B_], in0=dot[:, B_],
                              in1=tmp3[:, 2, B_], op=OP.add)

            dot_b = dot[:, B_].unsqueeze(1).broadcast_to([128, 3, nq_])
            eng.tensor_tensor(out=xraw[:, :, B_], in0=dot_b, in1=n_u, op=OP.mult)
            eng.tensor_tensor(out=xraw[:, :, B_], in0=a_u, in1=xraw[:, :, B_],
                              op=OP.subtract)
            eng.tensor_tensor(out=sqx[:, :, B_], in0=xraw[:, :, B_],
                              in1=xraw[:, :, B_], op=OP.mult)
            eng.tensor_tensor(out=ssx[:, B_], in0=sqx[:, 0, B_],
                              in1=sqx[:, 1, B_], op=OP.add)
            eng.tensor_tensor(out=ssx[:, B_], in0=ssx[:, B_],
                              in1=sqx[:, 2, B_], op=OP.add)
            nc.scalar.activation(nrmx[:, B_], ssx[:, B_], AF.Sqrt)
            eng.tensor_scalar_add(out=nrmx[:, B_], in0=nrmx[:, B_], scalar1=EPS)
            nc.vector.reciprocal(invx[:, B_], nrmx[:, B_])
            invx_b = invx[:, B_].unsqueeze(1).broadcast_to([128, 3, nq_])
            eng.tensor_tensor(out=x_u[:, :, B_], in0=xraw[:, :, B_], in1=invx_b,
                              op=OP.mult)

            for x_ in range(3):
                i1, i2 = (x_ + 1) % 3, (x_ + 2) % 3
                eng.tensor_tensor(out=yax[:, x_, B_], in0=n_u[:, i1, :],
                                  in1=x_u[:, i2, B_], op=OP.mult)
                eng.tensor_tensor(out=tmp3[:, x_, B_], in0=n_u[:, i2, :],
                                  in1=x_u[:, i1, B_], op=OP.mult)
            eng.tensor_tensor(out=yax[:, :, B_], in0=yax[:, :, B_],
                              in1=tmp3[:, :, B_], op=OP.subtract)

            eng.tensor_copy(out=outa_sb[:, :, b, ql:qh], in_=a_u)

        def emit_rotation(eng, b, q0, nq):
            """aligned geo (c' = 3i+t) for q in [q0, q0+nq) of batch b on `eng`.
            svec rows: i=0 -> |xraw| (nrmx), i=1 -> 0, i=2 -> dot.
            i emitted in order [2, 0, 1]: n_u is ready before x_u/yax."""
            qs = slice(b * NQ + q0, b * NQ + q0 + nq)
            qsl = slice(q0, q0 + nq)
            rrows = [x_u, yax, u2[:, :, 0, :]]
            svs = [nrmx, zero, dot]
            alq = al_pool.tile([128, nq, 3, 3, S], F16, name=f"alq_{b}_{q0}")
            for i in (2, 0, 1):
                out3 = alq[:, :, i]
                for j in range(3):
                    rb = rrows[i][:, j, qs].unsqueeze(2).unsqueeze(3) \
                        .broadcast_to([128, nq, 3, S])
                    src = geo_pt[b][:, qsl, j]
                    if j == 0:
                        eng.tensor_tensor(out=out3, in0=src, in1=rb, op=OP.mult)
                    else:
                        t = tmp_pool.tile([128, nq, 3, S], F32, tag=f"rtmp{nq}")
                        eng.tensor_tensor(out=t[:], in0=src, in1=rb, op=OP.mult)
                        eng.tensor_tensor(out=out3, in0=out3, in1=t[:], op=OP.add)
                dir_row = alq[:, :, i, 2, :]
                sv_b = svs[i][:, qs].unsqueeze(2).broadcast_to([128, nq, S])
                eng.tensor_tensor(out=dir_row, in0=sv_b, in1=dir_row,
                                  op=OP.subtract)
            return alq

        def emit_rt_write(b, alq, q0, nq, eng):
            with tc.high_priority():
                eng.dma_start(
                    out=geoT_d[b, q0:q0 + nq].rearrange("q c p s -> p q c s"),
                    in_=alq[:].rearrange("p q i f s -> p q (i f) s"),
                )

        def emit_rt_read(b, q0, nq, eng):
            with tc.high_priority():
                eng.dma_start(
                    out=xts[b][64:73, q0:q0 + nq],
                    in_=geoT_d[b, q0:q0 + nq].rearrange(
                        "q c p s -> c q (p s)", p=128, s=S),
                )

        # ================= startup emission =================
        # gpsimd queue order matters (strict FIFO): keep bulk-load emissions
        # that are not immediately needed BEHIND the rt round trips.
        emit_geo_load(0)
        emit_geo_load(1)
        emit_feats_load(0, 0, 2)
        emit_feats_load(0, 2, 2)
        emit_feats_load(0, 4, 4)
        emit_feats_load(1, 0, 4)
        emit_feats_load(1, 4, 4)

        emit_azi(0)
        emit_rphase(0, nc.vector)
        emit_azi(1)
        # b0 g0/g1 rotation on DVE (idle during startup)
        alq_00 = emit_rotation(nc.vector, 0, 0, 2)
        # q0's round trip in p-halves: chunk 0 only needs points 0-63, so
        # its gate lands one half-transfer earlier
        with tc.high_priority():
            nc.sync.dma_start(
                out=geoT_d[0, 0:1, :, 0:64, :].rearrange("q c p s -> p q c s"),
                in_=alq_00[0:64, 0:1].rearrange("p q i f s -> p q (i f) s"))
            nc.sync.dma_start(
                out=xts[0][64:73, 0:1].rearrange("c q (p s) -> c q p s", p=128, s=S)[:, :, 0:64],
                in_=geoT_d[0, 0:1, :, 0:64, :].rearrange("q c p s -> c q (p s)"))
            nc.sync.dma_start(
                out=geoT_d[0, 0:1, :, 64:128, :].rearrange("q c p s -> p q c s"),
                in_=alq_00[64:128, 0:1].rearrange("p q i f s -> p q (i f) s"))
            nc.sync.dma_start(
                out=xts[0][64:73, 0:1].rearrange("c q (p s) -> c q p s", p=128, s=S)[:, :, 64:128],
                in_=geoT_d[0, 0:1, :, 64:128, :].rearrange("q c p s -> c q (p s)"))
        with tc.high_priority():
            nc.scalar.dma_start(
                out=geoT_d[0, 1:2, :, 0:64, :].rearrange("q c p s -> p q c s"),
                in_=alq_00[0:64, 1:2].rearrange("p q i f s -> p q (i f) s"))
            nc.scalar.dma_start(
                out=xts[0][64:73, 1:2].rearrange("c q (p s) -> c q p s", p=128, s=S)[:, :, 0:64],
                in_=geoT_d[0, 1:2, :, 0:64, :].rearrange("q c p s -> c q (p s)"))
            nc.scalar.dma_start(
                out=geoT_d[0, 1:2, :, 64:128, :].rearrange("q c p s -> p q c s"),
                in_=alq_00[64:128, 1:2].rearrange("p q i f s -> p q (i f) s"))
            nc.scalar.dma_start(
                out=xts[0][64:73, 1:2].rearrange("c q (p s) -> c q p s", p=128, s=S)[:, :, 64:128],
                in_=geoT_d[0, 1:2, :, 64:128, :].rearrange("q c p s -> c q (p s)"))

        # b1 R phase: elementwise on gpsimd, sqrt on ACT, reciprocal on DVE
        emit_rphase(1, nc.gpsimd)

        alq_02 = emit_rotation(nc.vector, 0, 2, 2)
        emit_rt_write(0, alq_02, 2, 2, nc.sync)
        emit_rt_read(0, 2, 2, nc.sync)

        # PE warmup on rot-g0 output, extended by a second burst gated on
        # rot-g1 so HAM stays at 8/8 across the rt round-trip gap to chunk 0
        warm_ps0 = ps1_pool.tile([128, 1024], F32, tag="h1ps")
        warm_rhs = alq_00[:].rearrange("p q i f s -> p (q i f s)")
        for _ in range(18):
            nc.tensor.matmul(out=warm_ps0[:, 0:512], lhsT=w2T[:],
                             rhs=warm_rhs[:, 0:512], start=True, stop=True)


        # b0 rotation tail on gpsimd
        for (b, q0, nq) in ((0, 4, 2), (0, 6, 2)):
            alq = emit_rotation(nc.gpsimd, b, q0, nq)
            emit_rt_write(b, alq, q0, nq, nc.sync)
            emit_rt_read(b, q0, nq, nc.sync)
        # b1 rotation in nq=2 groups so each rt round trip starts ~9us
        # earlier than the half-batch version
        for q0 in (0, 2, 4, 6):
            alq = emit_rotation(nc.gpsimd, 1, q0, 2)
            emit_rt_write(1, alq, q0, 2, nc.sync)
            emit_rt_read(1, q0, 2, nc.sync)

        pooled = {}
        pooled_raw = {}
        for b in range(BL):
            pooled[b] = pb_pool.tile([128, P], F16, name=f"pooled_{b}")
            pooled_raw[b] = pb_pool.tile([128, P], F16, name=f"pooledr_{b}")

        # ================= chunk loop =================
        # Software-pipelined: PE stream is mm1(0), mm1(1), mm2(0), mm1(2), ...
        chunks = [(b, k) for b in range(BL) for k in range(NCH)]

        def emit_mm1(idx):
            b, k = chunks[idx]
            ql, j = k // 4, k % 4
            base = j * 1024
            xt = xts[b]
            h1ps = ps1_pool.tile([128, 1024], F32, tag="h1ps")
            m0 = nc.tensor.matmul(out=h1ps[:, 0:512], lhsT=w1c[:],
                                  rhs=xt[:, ql, base:base + 512],
                                  start=True, stop=True)
            m1 = nc.tensor.matmul(out=h1ps[:, 512:1024], lhsT=w1c[:],
                                  rhs=xt[:, ql, base + 512:base + 1024],
                                  start=True, stop=True)
            if SKIP_DUP_LDW:
                m1.ins.ldweights = False
            return h1ps

        h1ps_cur = emit_mm1(0)
        for idx, (b, k) in enumerate(chunks):
            h1sb = h1_pool.tile([128, 1024], F16, tag="h1sb")
            nc.scalar.activation(h1sb[:], h1ps_cur[:], AF.Relu, bias=b1t[:, 0:1])
            if idx + 1 < len(chunks):
                h1ps_cur = emit_mm1(idx + 1)
            h2ps = ps2_pool.tile([128, 1024], F32, tag="h2ps")
            m0 = nc.tensor.matmul(out=h2ps[:, 0:512], lhsT=w2T[:],
                                  rhs=h1sb[:, 0:512], start=True, stop=True)
            m1 = nc.tensor.matmul(out=h2ps[:, 512:1024], lhsT=w2T[:],
                                  rhs=h1sb[:, 512:1024], start=True, stop=True)
            if SKIP_DUP_LDW:
                m1.ins.ldweights = False
            po = k * 32
            nc.vector.reduce_max(
                out=pooled_raw[b][:, po:po + 32],
                in_=h2ps[:].rearrange("m (p s) -> m p s", s=S),
                axis=AX.X)
            last_seg = b == BL - 1 and k >= NCH - 8
            if last_seg and (k == NCH - 5 or k == NCH - 1):
                seg = slice(po + 32 - 128, po + 32)
                nc.scalar.activation(pooled[b][:, seg], pooled_raw[b][:, seg],
                                     AF.Relu, bias=b2t[:, 0:1])
                nc.sync.dma_start(out=outp_d[b, :, seg], in_=pooled[b][:, seg])
            elif not last_seg and k % 8 == 7:
                seg = slice(po + 32 - 256, po + 32)
                nc.scalar.activation(pooled[b][:, seg], pooled_raw[b][:, seg],
                                     AF.Relu, bias=b2t[:, 0:1])
                nc.sync.dma_start(out=outp_d[b, :, seg], in_=pooled[b][:, seg])

        nc.scalar.dma_start(out=outa_d[:, :, :, :], in_=outa_sb[:])

    nc.finalize()
    return nc


_CACHE = {}


def _get_program():
    if "nc" not in _CACHE:
        _CACHE["nc"] = build_program()
    return _CACHE["nc"]


def make_in_maps(input, normal, w1, b1, w2, b2):
    input = np.asarray(input, dtype=np.float32)
    normal = np.asarray(normal, dtype=np.float32)
    w1 = np.asarray(w1, dtype=np.float32)
    b1 = np.asarray(b1, dtype=np.float32)
    w2 = np.asarray(w2, dtype=np.float32)
    b2 = np.asarray(b2, dtype=np.float32)

    w1fT = w1[:, 3:67].T.astype(np.float16)
    w1gT = w1[:, GEO_W1_COLS].T.astype(np.float16)
    w1c = np.ascontiguousarray(np.concatenate([w1fT, w1gT], axis=0))
    w2T = np.ascontiguousarray(w2.T.astype(np.float16))
    b1c = np.ascontiguousarray(b1.reshape(128, 1))
    b2c = np.ascontiguousarray(b2.reshape(128, 1))

    in_maps = []
    for core in range(NCORES):
        b0 = core * BL
        inp = input[b0:b0 + BL]
        f = inp[:, 12:76].astype(np.float16)
        feats = np.ascontiguousarray(f.reshape(BL, 64, NQ, 4096))
        g = inp[:, 3:12].astype(np.float16)
        g = g.reshape(BL, 3, 3, NQ, 128, S).transpose(0, 4, 3, 2, 1, 5)
        geo = np.ascontiguousarray(g)
        # normp [128, 3, BL, NQ]
        normp = np.ascontiguousarray(
            normal[b0:b0 + BL].reshape(BL, NQ, 128, 3).transpose(2, 3, 0, 1))
        in_maps.append({
            "feats": feats, "geo": geo, "normp": normp,
            "w1c": w1c, "w2T": w2T, "b1c": b1c, "b2c": b2c,
            "geot": np.zeros((BL, NQ, 9, 128, S), np.float16),
        })
    return in_maps


def assemble_output(results):
    outs = []
    for r in results:
        outp = r["outp"].astype(np.float32)   # (BL,128,P)
        outa = r["outa"]                      # (128,3,BL,NQ)
        azi = outa.transpose(2, 1, 3, 0).reshape(BL, 3, P)
        outs.append(np.concatenate([azi, outp], axis=1))
    return np.concatenate(outs, axis=0)


def kernel(input, normal, w1, b1, w2, b2, _trace=False):
    nc = _get_program()
    in_maps = make_in_maps(input, normal, w1, b1, w2, b2)
    res = run_bass_kernel_spmd(nc, in_maps, core_ids=list(range(NCORES)), trace=_trace)
    out = assemble_output(res.results)
    if _trace:
        return out, res
    return out
